# revision 2
# baseline (speedup 1.0000x reference)
"""AUGRU (DIEN) Trainium2 Bass kernel, v3 "folded" layout.

Full-input contract: kernel(**inputs) takes the complete un-sharded arrays
(B=4096, T=200, D=U=64) and returns the full [4096, 64] fp32 output.

Pure data parallelism over 8 cores; within a core the 512 batch rows are
FOLDED two-per-partition-block: h_f[p, c] holds unit u=p%64 of batch rank
j=2c+(p//64) (rows sorted by length desc, dealt round-robin to cores), so
every elementwise tensor is [128, W<=256] and all 128 engine lanes are used.
Per-step active folded width W_t shrinks with the ragged lengths; alpha*mask
folds the mask so frozen tails are exact (q=0 -> h'=h).

Per step, per column group (two groups pipeline the serial chain):
  PE : p_zr[z|r] = blkdiag(0.2Wx_zr)@x_f + blkdiag(0.2Wr_zr)@h_f,
       p_rh = blkdiag(Wr_h)@h_f, p_h2 = blkdiag(Wx_h)@x_f + I@m
  ACT: t_zr = relu(p_zr + bias(0.5)) fp16 ; hh = tanh(p_h2 + b_h)
  GPS: q = min(t_z,1) * a_f                (scalar_tensor_tensor)
  DVE: m = min(t_r,1) * p_rh (STT) ; d = hh - h ; e = q*d ; h += e
"""
import sys
sys.path.insert(0, '/opt/trn_rl_repo')
from contextlib import ExitStack

import numpy as np

import concourse.bass as bass
import concourse.tile as tile
from concourse import bacc, mybir

F16 = mybir.dt.float16
F32 = mybir.dt.float32
Alu = mybir.AluOpType
Act = mybir.ActivationFunctionType

N_CORES = 8
B = 4096
T = 200
D = 64
U = 64
B_LOC = B // N_CORES   # 512
WF = B_LOC // 2        # 256 folded columns

# weight-set indices in io["wts"]
W_ZX, W_ZR, W_RX, W_RR, W_HX, W_HR, W_EYE = range(7)


def _declare_io(nc):
    io = {}
    io["xf"] = nc.dram_tensor("xf", [T, 128, WF], F16, kind="ExternalInput").ap()
    io["af"] = nc.dram_tensor("af", [T, 128, WF], F16, kind="ExternalInput").ap()
    io["wts"] = nc.dram_tensor("wts", [7, 128, 128], F16, kind="ExternalInput").ap()
    io["b_zr"] = nc.dram_tensor("b_zr", [128, 1], F32, kind="ExternalInput").ap()
    io["b_h"] = nc.dram_tensor("b_h", [128, 1], F32, kind="ExternalInput").ap()
    io["out"] = nc.dram_tensor("out", [128, WF], F16, kind="ExternalOutput").ap()
    return io


def _groups(w):
    """Split folded width w into two balanced 32-aligned column groups."""
    if w <= 32:
        return [(0, w)]
    g0 = min(((w + 1) // 2 + 31) // 32 * 32, w)
    if w - g0 <= 0:
        return [(0, w)]
    return [(0, g0), (g0, w)]


def _build_kernel(nc, tc, w_list=None, t_run=None, repeats=1):
    if w_list is None:
        w_list = [WF] * T
    if t_run is None:
        t_run = T
    io = _declare_io(nc)
    ctx = ExitStack()
    with ctx:
        const_pool = ctx.enter_context(tc.tile_pool(name="const", bufs=1))
        state_pool = ctx.enter_context(tc.tile_pool(name="state", bufs=1))
        xin_pool = ctx.enter_context(tc.tile_pool(name="xin", bufs=3))
        work_pool = ctx.enter_context(tc.tile_pool(name="work", bufs=4))
        ps_zr = ctx.enter_context(tc.tile_pool(name="ps_zr", bufs=2, space="PSUM"))
        ps_rhh = ctx.enter_context(tc.tile_pool(name="ps_rhh", bufs=2, space="PSUM"))

        wts = const_pool.tile([128, 7 * 128], F16)
        for _wi in range(7):
            nc.sync.dma_start(wts[:, 128 * _wi : 128 * (_wi + 1)], io["wts"][_wi])
        wt = [wts[:, 128 * i : 128 * (i + 1)] for i in range(7)]
        b_zr = const_pool.tile([128, 1], F32)
        nc.sync.dma_start(b_zr[:], io["b_zr"][:])
        b_h = const_pool.tile([128, 1], F32)
        nc.sync.dma_start(b_h[:], io["b_h"][:])

        h = state_pool.tile([128, WF], F16)

        for _rep in range(repeats):
          nc.vector.memset(h[:], 0.0)
          for t in range(t_run):
            w = w_list[t]
            phase = t % 2
            if phase == 0:
                w2 = max(w, w_list[t + 1]) if t + 1 < t_run else w
                xf = xin_pool.tile([128, 2 * WF], F16, tag="xf")
                nc.sync.dma_start(
                    xf[:].rearrange("p (s c) -> p s c", s=2)[:, :, 0:w2],
                    io["xf"][t : t + 2, :, 0:w2].rearrange("t p c -> p t c"),
                )
                af = xin_pool.tile([128, 2 * WF], F16, tag="af")
                nc.sync.dma_start(
                    af[:].rearrange("p (s c) -> p s c", s=2)[:, :, 0:w2],
                    io["af"][t : t + 2, :, 0:w2].rearrange("t p c -> p t c"),
                )
            xv = xf[:, phase * WF : phase * WF + WF]
            av = af[:, phase * WF : phase * WF + WF]

            groups = _groups(w)
            pz, pr, ph = [], [], []
            # --- stage 1: matmuls (weights cycle; reused across groups) ---
            for gi, (c0, c1) in enumerate(groups):
                gw = c1 - c0
                cs = slice(c0, c1)
                p_zr = ps_zr.tile([128, 256], F32, tag=f"zr{gi}")
                p_rhh = ps_rhh.tile([128, 256], F32, tag=f"rhh{gi}")
                pz.append(p_zr)
                pr.append(p_rhh[:, 0:128])
                ph.append(p_rhh[:, 128:256])
                zs, rs = slice(0, gw), slice(gw, 2 * gw)
                # PSUM rule: start=True clears has_written BANK-wide, so each
                # region's start..stop pair must complete before the next
                # start on the same bank; rh-mm precedes hx so the open
                # hx..inject pair is the bank's last accumulation chain.
                nc.tensor.matmul(p_zr[:, zs], wt[W_ZX], xv[:, cs],
                                 start=True, stop=False, skip_group_check=True)
                nc.tensor.matmul(p_zr[:, zs], wt[W_ZR], h[:, cs],
                                 start=False, stop=True, skip_group_check=True)
                nc.tensor.matmul(p_zr[:, rs], wt[W_RX], xv[:, cs],
                                 start=True, stop=False, skip_group_check=True)
                nc.tensor.matmul(p_zr[:, rs], wt[W_RR], h[:, cs],
                                 start=False, stop=True, skip_group_check=True)
                nc.tensor.matmul(pr[gi][:, 0:gw], wt[W_HR], h[:, cs],
                                 start=True, stop=True, skip_group_check=True)
                nc.tensor.matmul(ph[gi][:, 0:gw], wt[W_HX], xv[:, cs],
                                 start=True, stop=False, skip_group_check=True)

            t_zr_t = work_pool.tile([128, 512], F16, tag="t_zr")
            m_t = work_pool.tile([128, WF], F16, tag="m")
            q1_t = work_pool.tile([128, WF], F16, tag="q1")
            q_t = work_pool.tile([128, WF], F16, tag="q")
            w1_t = work_pool.tile([128, WF], F16, tag="w1")
            u_t = work_pool.tile([128, WF], F16, tag="u")
            v_t = work_pool.tile([128, WF], F16, tag="v")
            hh_t = work_pool.tile([128, WF], F16, tag="hh")

            # --- stage 2+: per-group gate chain ---
            for gi, (c0, c1) in enumerate(groups):
                gw = c1 - c0
                cs = slice(c0, c1)
                tz = t_zr_t[:, gi * 256 : gi * 256 + gw]
                tr = t_zr_t[:, gi * 256 + gw : gi * 256 + 2 * gw]
                tzr = t_zr_t[:, gi * 256 : gi * 256 + 2 * gw]
                nc.scalar.activation(tzr, pz[gi][:, 0 : 2 * gw], Act.Relu,
                                     bias=b_zr[:])
                # m = min(r',1) * rh
                nc.vector.scalar_tensor_tensor(
                    m_t[:, cs], tr, 1.0, pr[gi][:, 0:gw], Alu.min, Alu.mult)
                # q = min(z',1) * a
                nc.vector.tensor_scalar_min(q1_t[:, cs], tz, 1.0)
                nc.gpsimd.tensor_mul(q_t[:, cs], q1_t[:, cs], av[:, cs])
                # p_h2 += m
                nc.tensor.matmul(ph[gi][:, 0:gw], wt[W_EYE], m_t[:, cs],
                                 start=False, stop=True, skip_group_check=True)
                nc.scalar.activation(hh_t[:, cs], ph[gi][:, 0:gw], Act.Tanh,
                                     bias=b_h[:])
                # h' = (1-q) h + q hh
                nc.vector.tensor_scalar(w1_t[:, cs], q_t[:, cs], -1.0, 1.0,
                                        Alu.mult, Alu.add)
                nc.vector.tensor_mul(u_t[:, cs], w1_t[:, cs], h[:, cs])
                nc.vector.tensor_mul(v_t[:, cs], q_t[:, cs], hh_t[:, cs])
                nc.vector.tensor_add(h[:, cs], u_t[:, cs], v_t[:, cs])

        nc.sync.dma_start(io["out"][:], h[:])
    return io


_CACHE = {}


def _plan_widths(lengths):
    """Per-step active folded width per core (multiple of 64)."""
    lengths = np.asarray(lengths)
    n_t = (lengths[:, None] > np.arange(T)[None, :]).sum(0)
    per_core = np.ceil(n_t / N_CORES)
    wf = np.ceil(per_core / 2.0)
    wf = (np.ceil(wf / 64.0) * 64).astype(int)
    wf = np.maximum(wf, 64)
    wf = np.minimum(wf, WF)
    wf = np.maximum.accumulate(wf[::-1])[::-1]
    for i in range(0, T - 1, 2):
        wf[i + 1] = wf[i]
    return [int(x) for x in wf]


def _get_compiled(w_key, t_run=None, repeats=1):
    key = (w_key, t_run, repeats)
    if key not in _CACHE:
        nc = bacc.Bacc("TRN2", target_bir_lowering=False, num_devices=N_CORES)
        with tile.TileContext(nc) as tc:
            _build_kernel(nc, tc, list(w_key), t_run=t_run, repeats=repeats)
        nc.compile()
        _CACHE[key] = {"nc": nc}
    return _CACHE[key]


def host_prep(inputs, alphas, mask, kernel, recurrent_kernel, bias):
    """Sort rows by length desc, deal round-robin to cores, fold 2/partition-
    block, pack blkdiag weights.  Returns (in_maps, w_list, order)."""
    Wx = np.asarray(kernel, np.float32)
    Wr = np.asarray(recurrent_kernel, np.float32)
    bias = np.asarray(bias, np.float32)
    b_in, b_rec = bias[0], bias[1]

    def blk(a):
        o = np.zeros((128, 128), np.float32)
        o[0:64, 0:64] = a
        o[64:128, 64:128] = a
        return o

    wts = np.stack([
        blk(0.2 * Wx[:, 0:64]),
        blk(0.2 * Wr[:, 0:64]),
        blk(0.2 * Wx[:, 64:128]),
        blk(0.2 * Wr[:, 64:128]),
        blk(Wx[:, 128:192]),
        blk(Wr[:, 128:192]),
        np.eye(128, dtype=np.float32),
    ]).astype(np.float16)

    bz = 0.2 * (b_in + b_rec)[0:64] + 0.5
    br = 0.2 * (b_in + b_rec)[64:128] + 0.5
    assert np.allclose(bz, br), "per-gate z/r bias mismatch unsupported"
    assert np.allclose(b_rec[128:192], 0.0), "nonzero recurrent h-bias unsupported"
    b_zr = np.tile(bz, 2).reshape(128, 1).astype(np.float32)
    b_h = np.tile(b_in[128:192], 2).reshape(128, 1).astype(np.float32)

    mask = np.asarray(mask)
    lengths = mask.sum(1).astype(np.int64)
    order = np.argsort(-lengths, kind="stable")
    w_list = _plan_widths(lengths)

    at = (np.asarray(alphas, np.float32) * mask.astype(np.float32))  # [B, T]
    x_full = np.asarray(inputs, np.float32)

    common = {"wts": wts, "b_zr": b_zr, "b_h": b_h}
    in_maps = []
    for c in range(N_CORES):
        rows = order[c::N_CORES]          # 512 rows, sorted by length desc
        # rank j -> block j%2, folded col j//2
        xs = x_full[rows]                 # [512, T, 64]
        xs = xs.reshape(WF, 2, T, 64)     # [col, block, T, d]
        xf = np.ascontiguousarray(
            xs.transpose(2, 1, 3, 0).reshape(T, 128, WF)).astype(np.float16)
        ats = at[rows].reshape(WF, 2, T)  # [col, block, T]
        af = np.ascontiguousarray(
            np.broadcast_to(ats.transpose(2, 1, 0)[:, :, None, :],
                            (T, 2, 64, WF)).reshape(T, 128, WF)
        ).astype(np.float16)
        mcore = dict(common)
        mcore["xf"] = xf
        mcore["af"] = af
        in_maps.append(mcore)
    return in_maps, w_list, order


def _get_executor(w_key, t_run=None, repeats=1):
    """Build (once per width plan) a cached sharded jit callable."""
    entry = _get_compiled(w_key, t_run, repeats)
    if "exec" in entry:
        return entry["exec"]
    import jax
    from jax.experimental.shard_map import shard_map
    from jax.sharding import Mesh, PartitionSpec
    from concourse import bass2jax, mybir as mb

    nc = entry["nc"]
    bass2jax.install_neuronx_cc_hook()

    partition_name = nc.partition_id_tensor.name if nc.partition_id_tensor else None
    in_names, out_names, out_avals = [], [], []
    for alloc in nc.m.functions[0].allocations:
        if not isinstance(alloc, mb.MemoryLocationSet):
            continue
        name = alloc.memorylocations[0].name
        if alloc.kind == "ExternalInput":
            if name != partition_name:
                in_names.append(name)
        elif alloc.kind == "ExternalOutput":
            out_names.append(name)
            out_avals.append(
                jax.core.ShapedArray(tuple(alloc.tensor_shape), mb.dt.np(alloc.dtype))
            )
    n_params = len(in_names)
    all_in_names = list(in_names) + list(out_names)
    if partition_name is not None:
        all_in_names.append(partition_name)

    def _body(*args):
        operands = list(args)
        if partition_name is not None:
            operands.append(bass2jax.partition_id_tensor())
        outs = bass2jax._bass_exec_p.bind(
            *operands,
            out_avals=tuple(out_avals),
            in_names=tuple(all_in_names),
            out_names=tuple(out_names),
            lowering_input_output_aliases=(),
            sim_require_finite=True,
            sim_require_nnan=True,
            nc=nc,
        )
        return tuple(outs)

    devices = jax.devices()[:N_CORES]
    mesh = Mesh(np.asarray(devices), ("core",))
    n_outs = len(out_names)
    sharded = jax.jit(
        shard_map(
            _body,
            mesh=mesh,
            in_specs=(PartitionSpec("core"),) * (n_params + n_outs),
            out_specs=(PartitionSpec("core"),) * n_outs,
            check_rep=False,
        ),
        donate_argnums=tuple(range(n_params, n_params + n_outs)),
        keep_unused=True,
    )
    entry["exec"] = (sharded, in_names, out_names, out_avals, mesh)
    return entry["exec"]


def _run(in_maps, w_key):
    sharded, in_names, out_names, out_avals, _ = _get_executor(w_key)
    concat_in = [
        np.concatenate([np.asarray(in_maps[c][n]) for c in range(N_CORES)], axis=0)
        for n in in_names
    ]
    concat_zeros = [
        np.zeros((N_CORES * a.shape[0], *a.shape[1:]), a.dtype) for a in out_avals
    ]
    out_arrs = sharded(*concat_in, *concat_zeros)
    return {
        n: np.asarray(out_arrs[i]).reshape(N_CORES, *out_avals[i].shape)
        for i, n in enumerate(out_names)
    }


def bench(in_maps, w_key, iters=8, t_run=None, repeats=1):
    """Time device-side executions with inputs resident on device."""
    import time as _time
    import jax
    from jax.sharding import NamedSharding, PartitionSpec
    sharded, in_names, out_names, out_avals, mesh = _get_executor(w_key, t_run, repeats)
    sh = NamedSharding(mesh, PartitionSpec("core"))
    dev_in = [
        jax.device_put(
            np.concatenate([np.asarray(in_maps[c][n]) for c in range(N_CORES)], 0), sh
        )
        for n in in_names
    ]
    jax.block_until_ready(dev_in)
    times = []
    for _ in range(iters):
        zeros = [
            jax.device_put(
                np.zeros((N_CORES * a.shape[0], *a.shape[1:]), a.dtype), sh
            )
            for a in out_avals
        ]
        jax.block_until_ready(zeros)
        t0 = _time.time()
        out = sharded(*dev_in, *zeros)
        jax.block_until_ready(out)
        times.append(_time.time() - t0)
    return times


def kernel(inputs, alphas, mask, kernel, recurrent_kernel, bias):
    in_maps, w_list, order = host_prep(
        inputs, alphas, mask, kernel, recurrent_kernel, bias
    )
    outs = _run(in_maps, tuple(w_list))
    res = outs["out"]  # [N_CORES, 128, WF] fp16
    out = np.empty((B, U), np.float32)
    for c in range(N_CORES):
        rows = order[c::N_CORES]
        hf = res[c]                       # [128, WF]
        # rank j = 2c+block -> h = hf[block*64:(block+1)*64, col]
        hj = hf.reshape(2, 64, WF).transpose(2, 0, 1).reshape(B_LOC, 64).astype(np.float32)
        out[rows] = hj
    return out


# revision 4
# speedup vs baseline: 1.1778x; 1.1778x over previous
"""AUGRU (DIEN) Trainium2 Bass kernel, v3 "folded" layout.

Full-input contract: kernel(**inputs) takes the complete un-sharded arrays
(B=4096, T=200, D=U=64) and returns the full [4096, 64] fp32 output.

Pure data parallelism over 8 cores; within a core the 512 batch rows are
FOLDED two-per-partition-block: h_f[p, c] holds unit u=p%64 of batch rank
j=2c+(p//64) (rows sorted by length desc, dealt round-robin to cores), so
every elementwise tensor is [128, W<=256] and all 128 engine lanes are used.
Per-step active folded width W_t shrinks with the ragged lengths; alpha*mask
folds the mask so frozen tails are exact (q=0 -> h'=h).

Per step, per column group (two groups pipeline the serial chain):
  PE : p_zr[z|r] = blkdiag(0.2Wx_zr)@x_f + blkdiag(0.2Wr_zr)@h_f,
       p_rh = blkdiag(Wr_h)@h_f, p_h2 = blkdiag(Wx_h)@x_f + I@m
  ACT: t_zr = relu(p_zr + bias(0.5)) fp16 ; hh = tanh(p_h2 + b_h)
  GPS: q = min(t_z,1) * a_f                (scalar_tensor_tensor)
  DVE: m = min(t_r,1) * p_rh (STT) ; d = hh - h ; e = q*d ; h += e
"""
import sys
sys.path.insert(0, '/opt/trn_rl_repo')
from contextlib import ExitStack

import numpy as np

import concourse.bass as bass
import concourse.tile as tile
from concourse import bacc, mybir

F16 = mybir.dt.float16
F32 = mybir.dt.float32
Alu = mybir.AluOpType
Act = mybir.ActivationFunctionType

N_CORES = 8
B = 4096
T = 200
D = 64
U = 64
B_LOC = B // N_CORES   # 512
WF = B_LOC // 2        # 256 folded columns

# weight-set indices in io["wts"]
W_ZX, W_ZR, W_RX, W_RR, W_HX, W_HR, W_EYE = range(7)


def _declare_io(nc):
    io = {}
    io["xf"] = nc.dram_tensor("xf", [T, 128, WF], F16, kind="ExternalInput").ap()
    io["af"] = nc.dram_tensor("af", [T, 128, WF], F16, kind="ExternalInput").ap()
    io["wts"] = nc.dram_tensor("wts", [7, 128, 128], F16, kind="ExternalInput").ap()
    io["b_zr"] = nc.dram_tensor("b_zr", [128, 1], F32, kind="ExternalInput").ap()
    io["b_h"] = nc.dram_tensor("b_h", [128, 1], F32, kind="ExternalInput").ap()
    io["out"] = nc.dram_tensor("out", [128, WF], F16, kind="ExternalOutput").ap()
    return io


def _groups(w):
    """Split folded width w into two balanced 32-aligned column groups."""
    if w <= 32:
        return [(0, w)]
    g0 = min(((w + 1) // 2 + 31) // 32 * 32, w)
    if w - g0 <= 0:
        return [(0, w)]
    return [(0, g0), (g0, w)]


def _build_kernel(nc, tc, w_list=None, t_run=None, repeats=1):
    if w_list is None:
        w_list = [WF] * T
    if t_run is None:
        t_run = T
    io = _declare_io(nc)
    ctx = ExitStack()
    with ctx:
        const_pool = ctx.enter_context(tc.tile_pool(name="const", bufs=1))
        state_pool = ctx.enter_context(tc.tile_pool(name="state", bufs=1))
        xin_pool = ctx.enter_context(tc.tile_pool(name="xin", bufs=3))
        work_pool = ctx.enter_context(tc.tile_pool(name="work", bufs=4))
        ps_zr = ctx.enter_context(tc.tile_pool(name="ps_zr", bufs=2, space="PSUM"))
        ps_rhh = ctx.enter_context(tc.tile_pool(name="ps_rhh", bufs=2, space="PSUM"))

        wts = const_pool.tile([128, 7 * 128], F16)
        for _wi in range(7):
            nc.sync.dma_start(wts[:, 128 * _wi : 128 * (_wi + 1)], io["wts"][_wi])
        wt = [wts[:, 128 * i : 128 * (i + 1)] for i in range(7)]
        b_zr = const_pool.tile([128, 1], F32)
        nc.sync.dma_start(b_zr[:], io["b_zr"][:])
        b_h = const_pool.tile([128, 1], F32)
        nc.sync.dma_start(b_h[:], io["b_h"][:])

        h = state_pool.tile([128, WF], F16)

        for _rep in range(repeats):
          nc.vector.memset(h[:], 0.0)
          u_prev = v_prev = None
          for t in range(t_run):
            w = w_list[t]
            phase = t % 2
            if phase == 0:
                w2 = max(w, w_list[t + 1]) if t + 1 < t_run else w
                xf = xin_pool.tile([128, 2 * WF], F16, tag="xf")
                nc.sync.dma_start(
                    xf[:].rearrange("p (s c) -> p s c", s=2)[:, :, 0:w2],
                    io["xf"][t : t + 2, :, 0:w2].rearrange("t p c -> p t c"),
                )
                af = xin_pool.tile([128, 2 * WF], F16, tag="af")
                nc.sync.dma_start(
                    af[:].rearrange("p (s c) -> p s c", s=2)[:, :, 0:w2],
                    io["af"][t : t + 2, :, 0:w2].rearrange("t p c -> p t c"),
                )
            xv = xf[:, phase * WF : phase * WF + WF]
            av = af[:, phase * WF : phase * WF + WF]

            groups = _groups(w)
            pz, pr, ph = [], [], []
            # --- stage 1: matmuls (weights cycle; reused across groups) ---
            for gi, (c0, c1) in enumerate(groups):
                gw = c1 - c0
                cs = slice(c0, c1)
                p_zr = ps_zr.tile([128, 256], F32, tag=f"zr{gi}")
                p_rhh = ps_rhh.tile([128, 256], F32, tag=f"rhh{gi}")
                pz.append(p_zr)
                pr.append(p_rhh[:, 0:128])
                ph.append(p_rhh[:, 128:256])
                zs, rs = slice(0, gw), slice(gw, 2 * gw)
                # PSUM rule: start=True clears has_written BANK-wide, so each
                # region's start..stop pair must complete before the next
                # start on the same bank; rh-mm precedes hx so the open
                # hx..inject pair is the bank's last accumulation chain.
                # r-pair and rh first: they feed the serial chain
                # (relu_r -> STT_m -> inject -> tanh); z-pair only feeds the
                # off-chain q path.  For t>=1 the recurrent contribution is
                # expanded through h = u + v (Wr@h = Wr@u + Wr@v) so the
                # chain-critical mm consumes v(t-1) straight after its tanh,
                # without waiting for the h materialization.
                if u_prev is None:
                    rec = [h]          # h == 0 at t=0
                else:
                    rec = [u_prev, v_prev]
                nc.tensor.matmul(p_zr[:, rs], wt[W_RX], xv[:, cs],
                                 start=True, stop=False, skip_group_check=True)
                for ri, rsrc in enumerate(rec):
                    nc.tensor.matmul(p_zr[:, rs], wt[W_RR], rsrc[:, cs],
                                     start=False, stop=(ri == len(rec) - 1),
                                     skip_group_check=True)
                for ri, rsrc in enumerate(rec):
                    nc.tensor.matmul(pr[gi][:, 0:gw], wt[W_HR], rsrc[:, cs],
                                     start=(ri == 0), stop=(ri == len(rec) - 1),
                                     skip_group_check=True)
                nc.tensor.matmul(p_zr[:, zs], wt[W_ZX], xv[:, cs],
                                 start=True, stop=False, skip_group_check=True)
                for ri, rsrc in enumerate(rec):
                    nc.tensor.matmul(p_zr[:, zs], wt[W_ZR], rsrc[:, cs],
                                     start=False, stop=(ri == len(rec) - 1),
                                     skip_group_check=True)
                nc.tensor.matmul(ph[gi][:, 0:gw], wt[W_HX], xv[:, cs],
                                 start=True, stop=False, skip_group_check=True)

            t_zr_t = work_pool.tile([128, 512], F16, tag="t_zr")
            m_t = work_pool.tile([128, WF], F16, tag="m")
            q1_t = work_pool.tile([128, WF], F16, tag="q1")
            q_t = work_pool.tile([128, WF], F16, tag="q")
            w1_t = work_pool.tile([128, WF], F16, tag="w1")
            u_t = work_pool.tile([128, WF], F16, tag="u")
            v_t = work_pool.tile([128, WF], F16, tag="v")
            hh_t = work_pool.tile([128, WF], F16, tag="hh")

            # --- stage 2+: per-group gate chain ---
            for gi, (c0, c1) in enumerate(groups):
                gw = c1 - c0
                cs = slice(c0, c1)
                tz = t_zr_t[:, gi * 256 : gi * 256 + gw]
                tr = t_zr_t[:, gi * 256 + gw : gi * 256 + 2 * gw]
                # split relu: r-half first (it is on the serial chain)
                nc.scalar.activation(tr, pz[gi][:, gw : 2 * gw], Act.Relu,
                                     bias=b_zr[:])
                nc.scalar.activation(tz, pz[gi][:, 0:gw], Act.Relu,
                                     bias=b_zr[:])
                # m = min(r',1) * rh
                nc.vector.scalar_tensor_tensor(
                    m_t[:, cs], tr, 1.0, pr[gi][:, 0:gw], Alu.min, Alu.mult)
                # q = min(z',1) * a
                nc.vector.tensor_scalar_min(q1_t[:, cs], tz, 1.0)
                nc.gpsimd.tensor_mul(q_t[:, cs], q1_t[:, cs], av[:, cs])
                # p_h2 += m
                nc.tensor.matmul(ph[gi][:, 0:gw], wt[W_EYE], m_t[:, cs],
                                 start=False, stop=True, skip_group_check=True)
                nc.scalar.activation(hh_t[:, cs], ph[gi][:, 0:gw], Act.Tanh,
                                     bias=b_h[:])
                # h' = (1-q) h + q hh
                nc.vector.tensor_scalar(w1_t[:, cs], q_t[:, cs], -1.0, 1.0,
                                        Alu.mult, Alu.add)
                nc.vector.tensor_mul(u_t[:, cs], w1_t[:, cs], h[:, cs])
                nc.vector.tensor_mul(v_t[:, cs], q_t[:, cs], hh_t[:, cs])
                nc.vector.tensor_add(h[:, cs], u_t[:, cs], v_t[:, cs])
            u_prev, v_prev = u_t, v_t

        nc.sync.dma_start(io["out"][:], h[:])
    return io


_CACHE = {}


def _plan_widths(lengths):
    """Per-step active folded width per core (multiple of 64)."""
    lengths = np.asarray(lengths)
    n_t = (lengths[:, None] > np.arange(T)[None, :]).sum(0)
    per_core = np.ceil(n_t / N_CORES)
    wf = np.ceil(per_core / 2.0)
    wf = (np.ceil(wf / 64.0) * 64).astype(int)
    wf = np.maximum(wf, 64)
    wf = np.minimum(wf, WF)
    wf = np.maximum.accumulate(wf[::-1])[::-1]
    for i in range(0, T - 1, 2):
        wf[i + 1] = wf[i]
    return [int(x) for x in wf]


def _get_compiled(w_key, t_run=None, repeats=1):
    key = (w_key, t_run, repeats)
    if key not in _CACHE:
        nc = bacc.Bacc("TRN2", target_bir_lowering=False, num_devices=N_CORES)
        with tile.TileContext(nc) as tc:
            _build_kernel(nc, tc, list(w_key), t_run=t_run, repeats=repeats)
        nc.compile()
        _CACHE[key] = {"nc": nc}
    return _CACHE[key]


def host_prep(inputs, alphas, mask, kernel, recurrent_kernel, bias):
    """Sort rows by length desc, deal round-robin to cores, fold 2/partition-
    block, pack blkdiag weights.  Returns (in_maps, w_list, order)."""
    Wx = np.asarray(kernel, np.float32)
    Wr = np.asarray(recurrent_kernel, np.float32)
    bias = np.asarray(bias, np.float32)
    b_in, b_rec = bias[0], bias[1]

    def blk(a):
        o = np.zeros((128, 128), np.float32)
        o[0:64, 0:64] = a
        o[64:128, 64:128] = a
        return o

    wts = np.stack([
        blk(0.2 * Wx[:, 0:64]),
        blk(0.2 * Wr[:, 0:64]),
        blk(0.2 * Wx[:, 64:128]),
        blk(0.2 * Wr[:, 64:128]),
        blk(Wx[:, 128:192]),
        blk(Wr[:, 128:192]),
        np.eye(128, dtype=np.float32),
    ]).astype(np.float16)

    bz = 0.2 * (b_in + b_rec)[0:64] + 0.5
    br = 0.2 * (b_in + b_rec)[64:128] + 0.5
    assert np.allclose(bz, br), "per-gate z/r bias mismatch unsupported"
    assert np.allclose(b_rec[128:192], 0.0), "nonzero recurrent h-bias unsupported"
    b_zr = np.tile(bz, 2).reshape(128, 1).astype(np.float32)
    b_h = np.tile(b_in[128:192], 2).reshape(128, 1).astype(np.float32)

    mask = np.asarray(mask)
    lengths = mask.sum(1).astype(np.int64)
    order = np.argsort(-lengths, kind="stable")
    w_list = _plan_widths(lengths)

    at = (np.asarray(alphas, np.float32) * mask.astype(np.float32))  # [B, T]
    x_full = np.asarray(inputs, np.float32)

    common = {"wts": wts, "b_zr": b_zr, "b_h": b_h}
    in_maps = []
    for c in range(N_CORES):
        rows = order[c::N_CORES]          # 512 rows, sorted by length desc
        # rank j -> block j%2, folded col j//2
        xs = x_full[rows]                 # [512, T, 64]
        xs = xs.reshape(WF, 2, T, 64)     # [col, block, T, d]
        xf = np.ascontiguousarray(
            xs.transpose(2, 1, 3, 0).reshape(T, 128, WF)).astype(np.float16)
        ats = at[rows].reshape(WF, 2, T)  # [col, block, T]
        af = np.ascontiguousarray(
            np.broadcast_to(ats.transpose(2, 1, 0)[:, :, None, :],
                            (T, 2, 64, WF)).reshape(T, 128, WF)
        ).astype(np.float16)
        mcore = dict(common)
        mcore["xf"] = xf
        mcore["af"] = af
        in_maps.append(mcore)
    return in_maps, w_list, order


def _get_executor(w_key, t_run=None, repeats=1):
    """Build (once per width plan) a cached sharded jit callable."""
    entry = _get_compiled(w_key, t_run, repeats)
    if "exec" in entry:
        return entry["exec"]
    import jax
    from jax.experimental.shard_map import shard_map
    from jax.sharding import Mesh, PartitionSpec
    from concourse import bass2jax, mybir as mb

    nc = entry["nc"]
    bass2jax.install_neuronx_cc_hook()

    partition_name = nc.partition_id_tensor.name if nc.partition_id_tensor else None
    in_names, out_names, out_avals = [], [], []
    for alloc in nc.m.functions[0].allocations:
        if not isinstance(alloc, mb.MemoryLocationSet):
            continue
        name = alloc.memorylocations[0].name
        if alloc.kind == "ExternalInput":
            if name != partition_name:
                in_names.append(name)
        elif alloc.kind == "ExternalOutput":
            out_names.append(name)
            out_avals.append(
                jax.core.ShapedArray(tuple(alloc.tensor_shape), mb.dt.np(alloc.dtype))
            )
    n_params = len(in_names)
    all_in_names = list(in_names) + list(out_names)
    if partition_name is not None:
        all_in_names.append(partition_name)

    def _body(*args):
        operands = list(args)
        if partition_name is not None:
            operands.append(bass2jax.partition_id_tensor())
        outs = bass2jax._bass_exec_p.bind(
            *operands,
            out_avals=tuple(out_avals),
            in_names=tuple(all_in_names),
            out_names=tuple(out_names),
            lowering_input_output_aliases=(),
            sim_require_finite=True,
            sim_require_nnan=True,
            nc=nc,
        )
        return tuple(outs)

    devices = jax.devices()[:N_CORES]
    mesh = Mesh(np.asarray(devices), ("core",))
    n_outs = len(out_names)
    sharded = jax.jit(
        shard_map(
            _body,
            mesh=mesh,
            in_specs=(PartitionSpec("core"),) * (n_params + n_outs),
            out_specs=(PartitionSpec("core"),) * n_outs,
            check_rep=False,
        ),
        donate_argnums=tuple(range(n_params, n_params + n_outs)),
        keep_unused=True,
    )
    entry["exec"] = (sharded, in_names, out_names, out_avals, mesh)
    return entry["exec"]


def _run(in_maps, w_key):
    sharded, in_names, out_names, out_avals, _ = _get_executor(w_key)
    concat_in = [
        np.concatenate([np.asarray(in_maps[c][n]) for c in range(N_CORES)], axis=0)
        for n in in_names
    ]
    concat_zeros = [
        np.zeros((N_CORES * a.shape[0], *a.shape[1:]), a.dtype) for a in out_avals
    ]
    out_arrs = sharded(*concat_in, *concat_zeros)
    return {
        n: np.asarray(out_arrs[i]).reshape(N_CORES, *out_avals[i].shape)
        for i, n in enumerate(out_names)
    }


def bench(in_maps, w_key, iters=8, t_run=None, repeats=1):
    """Time device-side executions with inputs resident on device."""
    import time as _time
    import jax
    from jax.sharding import NamedSharding, PartitionSpec
    sharded, in_names, out_names, out_avals, mesh = _get_executor(w_key, t_run, repeats)
    sh = NamedSharding(mesh, PartitionSpec("core"))
    dev_in = [
        jax.device_put(
            np.concatenate([np.asarray(in_maps[c][n]) for c in range(N_CORES)], 0), sh
        )
        for n in in_names
    ]
    jax.block_until_ready(dev_in)
    times = []
    for _ in range(iters):
        zeros = [
            jax.device_put(
                np.zeros((N_CORES * a.shape[0], *a.shape[1:]), a.dtype), sh
            )
            for a in out_avals
        ]
        jax.block_until_ready(zeros)
        t0 = _time.time()
        out = sharded(*dev_in, *zeros)
        jax.block_until_ready(out)
        times.append(_time.time() - t0)
    return times


def kernel(inputs, alphas, mask, kernel, recurrent_kernel, bias):
    in_maps, w_list, order = host_prep(
        inputs, alphas, mask, kernel, recurrent_kernel, bias
    )
    outs = _run(in_maps, tuple(w_list))
    res = outs["out"]  # [N_CORES, 128, WF] fp16
    out = np.empty((B, U), np.float32)
    for c in range(N_CORES):
        rows = order[c::N_CORES]
        hf = res[c]                       # [128, WF]
        # rank j = 2c+block -> h = hf[block*64:(block+1)*64, col]
        hj = hf.reshape(2, 64, WF).transpose(2, 0, 1).reshape(B_LOC, 64).astype(np.float32)
        out[rows] = hj
    return out
